# revision 30
# baseline (speedup 1.0000x reference)
"""Trainium2 Bass kernel for 2-hop MixHop GCN (nn_Mixhop).

Strategy (8 NeuronCores, node sharding):
  h = x @ W1 (+b1);  GCN norm folded into row scales:
      g = dinv * h;  y[d] = dinv[d] * sum_{e: src->d} g[src]
  Per hop: the fp16 g-table is AllGathered in TWO pieces (A = local rows
  0..4095 of every core, B = rows 4096..8191) and the hop runs in two
  matching phases: phase A gathers+reduces all A-side edges into an SBUF
  partial (Apart), phase B re-injects the partial via an identity matmul
  and finishes with the B-side edges, so each ~90us collective piece
  hides behind the other phase's gather traffic.  Source rows are
  fetched per edge with gpsimd dma_gather (int16 indices address the
  32K-row A/B tables): one call per (window, half) rotated across the 4
  SWDGE queues, with each core's runtime edge count in a register so the
  trailing -1 padding idxs (chunk-ceil + max-over-cores slack, ~11% of
  descriptors) are skipped by the DGE.  Segment sums are PE matmuls
  against host-built one-hot fp8 "S" matrices (PSUM accumulation per
  128-dst window).  Self-loop edges never hit the gather path: their
  contribution dinv[d]^2*h[d] = dinv[d]*g[d] is injected by the same
  identity-matmul trick from the on-chip g tiles.  lin2 (@W2) streams
  per window at drain time into an SBUF logits accumulator; log_softmax
  at the end.

  DMA-ring discipline: the sync HWDGE ring carries only input loads
  (xt, S, weights - never blocked on compute), the scalar ring carries
  the g-table writes (queued right behind the scalar drains that
  produce them), and gathers live on the 4 SWDGE queues.
"""

import os
import sys

sys.path.insert(0, "/opt/trn_rl_repo")

import numpy as np

import concourse.bacc as bacc
import concourse.bass as bass
import concourse.mybir as mybir
import concourse.tile as tile
from concourse.bass_utils import run_bass_kernel_spmd

F32 = mybir.dt.float32
F16 = mybir.dt.float16
FP8 = mybir.dt.float8e4
I16 = mybir.dt.int16
NP_FP8 = mybir.dt.np(FP8)
NP_F16 = np.float16

N_CORES = 8
WIN = 128          # dst nodes per PSUM window
CHUNK = 128        # edges per matmul chunk
WG = 4             # windows per gather group
XB = 4             # windows per xt load

LAST_EXEC_NS = None
LAST_RESULTS = None


def _preprocess(x, edge_index, W1, b1, W2, b2):
    """Build the chunk plan (program-level constants, max over cores) and
    per-core input arrays."""
    n_nodes, d_in = x.shape
    hid = W1.shape[1]
    ncls = W2.shape[1]
    nmat = W2.shape[0] // hid
    assert n_nodes % (N_CORES * WIN) == 0
    NLOC = n_nodes // N_CORES
    NLH = NLOC // 2            # rows per A/B table piece per core
    NW = NLOC // WIN
    assert NW % WG == 0
    NG = NW // WG
    KIN = d_in // 128
    assert d_in % 128 == 0 and hid == 128

    src = np.asarray(edge_index[0], dtype=np.int64)
    dst = np.asarray(edge_index[1], dtype=np.int64)

    # degrees include the self loops even though loops bypass the gather
    deg = (np.bincount(dst, minlength=n_nodes) + 1).astype(np.float32)
    dinv = (1.0 / np.sqrt(deg)).astype(np.float32)

    core = dst // NLOC
    w_of = (dst % NLOC) // WIN
    s_r = src % NLOC
    half_of = (s_r >= NLH).astype(np.int64)   # 0 = table A, 1 = table B
    dloc = (dst % WIN).astype(np.int64)
    # int16 index into table A/B: core-major, NLH rows per core
    tab_idx = (src // NLOC) * NLH + (s_r - half_of * NLH)

    # counts per (core, window, half) -> program chunk counts = max over cores
    key = (core * NW + w_of) * 2 + half_of
    cnt = np.bincount(key, minlength=N_CORES * NW * 2).reshape(N_CORES, NW, 2)
    chunks_pc = -(-cnt // CHUNK)  # ceil-div per core
    C = chunks_pc.max(axis=0)     # [NW, 2] max over cores
    CA, CB = C[:, 0].copy(), C[:, 1].copy()
    TOTA, TOTB = int(CA.sum()), int(CB.sum())

    # per-phase group layouts: phase P columns are window-major
    NAg = np.array([CA[g * WG:(g + 1) * WG].sum() for g in range(NG)])
    NBg = np.array([CB[g * WG:(g + 1) * WG].sum() for g in range(NG)])
    gbaseA = np.concatenate([[0], np.cumsum(NAg)[:-1]])
    gbaseB = np.concatenate([[0], np.cumsum(NBg)[:-1]])
    MAXGH = int(max(NAg.max(), NBg.max()))

    colA = np.concatenate([[0], np.cumsum(CA)[:-1]])  # flat col base (A space)
    colB = np.concatenate([[0], np.cumsum(CB)[:-1]])  # flat col base (B space)

    # flat gather-slot base for (w, half): phase A slots then phase B slots
    slotbase = np.zeros((NW, 2), np.int64)
    for w in range(NW):
        slotbase[w, 0] = colA[w] * CHUNK
        slotbase[w, 1] = (TOTA + colB[w]) * CHUNK
    TOTC = TOTA + TOTB
    TOTSLOTS = TOTC * CHUNK

    one_fp8 = np.float32(1.0).astype(NP_FP8).view(np.uint8)

    plan = dict(
        n_nodes=n_nodes, NLOC=NLOC, NLH=NLH, NW=NW, NG=NG, KIN=KIN,
        hid=hid, ncls=ncls, nmat=nmat,
        CA=CA, CB=CB, NAg=NAg, NBg=NBg, gbaseA=gbaseA, gbaseB=gbaseB,
        colA=colA, colB=colB, TOTA=TOTA, TOTB=TOTB,
        TOTC=TOTC, TOTSLOTS=TOTSLOTS, MAXGH=MAXGH,
        has_b1=bool(np.any(b1 != 0)), has_b2=bool(np.any(b2 != 0)),
    )

    in_maps = []
    for p in range(N_CORES):
        sel = core == p
        t_p, w_p, h_p, dl_p = tab_idx[sel], w_of[sel], half_of[sel], dloc[sel]
        k = w_p * 2 + h_p
        order = np.argsort(k, kind="stable")
        ks = k[order]
        gcnt = np.bincount(ks, minlength=NW * 2)
        run_start = np.cumsum(gcnt) - gcnt
        run_pos = np.arange(len(ks)) - np.repeat(run_start, gcnt)
        slots = slotbase.reshape(-1)[ks] + run_pos

        idx_flat = np.full(TOTSLOTS, -1, np.int16)
        idx_flat[slots] = t_p[order].astype(np.int16)
        gcnt32 = gcnt.astype(np.int32).reshape(1, NW * 2)
        idx16 = idx_flat.reshape(TOTSLOTS // 16, 16).T  # [16, S/16]
        idx_arr = np.tile(idx16, (8, 1)).copy()         # [128, S/16]

        # S one-hot: row = pos-in-chunk, col = slot-chunk (same order as
        # the gather slots, so S col base == slot base / CHUNK * CHUNK)
        su8 = np.zeros((CHUNK, TOTC * CHUNK), np.uint8)
        pos = run_pos % CHUNK
        scol = slots - pos + dl_p[order]
        su8[pos, scol] = one_fp8
        s_arr = su8.view(NP_FP8)

        x_p = np.asarray(x[p * NLOC:(p + 1) * NLOC], dtype=np.float32)
        xt = np.ascontiguousarray(
            x_p.reshape(NW, 128, KIN, 128).transpose(0, 3, 2, 1)
            .reshape(NW // XB, XB, 128, KIN * 128).transpose(0, 2, 1, 3)
            .reshape(NW // XB, 128, XB * KIN * 128))
        dinv_p = np.ascontiguousarray(
            dinv[p * NLOC:(p + 1) * NLOC].reshape(NW, 128).T)

        m = {
            "xt": xt.astype(NP_F16),
            "w1": np.ascontiguousarray(
                np.asarray(W1, np.float32).reshape(KIN, 128, hid)
                .transpose(1, 0, 2).reshape(128, KIN * hid)).astype(NP_F16),
            "w2": np.ascontiguousarray(
                np.asarray(W2, np.float32).reshape(nmat, hid, ncls)
                .astype(NP_F16).transpose(1, 0, 2).reshape(hid, nmat * ncls)),
            "dinv": dinv_p,
            "dinv2": (dinv_p * dinv_p),
            "idx": idx_arr,
            "gcnt": gcnt32,
            "sdat": s_arr,
            "ident": np.eye(128, dtype=NP_F16),
        }
        if plan["has_b1"]:
            m["b1bc"] = np.tile(np.asarray(b1, np.float32)[None, :], (128, 1))
        if plan["has_b2"]:
            m["b2bc"] = np.tile(np.asarray(b2, np.float32)[None, :], (128, 1))
        in_maps.append(m)
    return plan, in_maps


def _build(plan):
    P = plan
    NLOC, NLH, NW, NG, KIN = P["NLOC"], P["NLH"], P["NW"], P["NG"], P["KIN"]
    HID, NCLS, NMAT = P["hid"], P["ncls"], P["nmat"]
    NN = P["n_nodes"]
    CA, CB = P["CA"], P["CB"]
    NAg, NBg = P["NAg"], P["NBg"]
    gbaseA, gbaseB = P["gbaseA"], P["gbaseB"]
    colA, colB, TOTA = P["colA"], P["colB"], P["TOTA"]
    MAXGH, TOTC, TOTSLOTS = P["MAXGH"], P["TOTC"], P["TOTSLOTS"]
    NGH = NG // 2   # groups per A/B table piece

    nc = bacc.Bacc("TRN2", target_bir_lowering=False, debug=False,
                   num_devices=N_CORES, num_swdge_queues=4)
    xt_d = nc.dram_tensor("xt", [NW // XB, 128, XB * KIN * 128], F16,
                          kind="ExternalInput")
    w1_d = nc.dram_tensor("w1", [128, KIN * HID], F16, kind="ExternalInput")
    w2_d = nc.dram_tensor("w2", [128, NMAT * NCLS], F16, kind="ExternalInput")
    dinv_d = nc.dram_tensor("dinv", [128, NW], F32, kind="ExternalInput")
    dinv2_d = nc.dram_tensor("dinv2", [128, NW], F32, kind="ExternalInput")
    idx_d = nc.dram_tensor("idx", [128, TOTSLOTS // 16], I16,
                           kind="ExternalInput")
    gcnt_d = nc.dram_tensor("gcnt", [1, NW * 2], mybir.dt.int32,
                            kind="ExternalInput")
    sdat_d = nc.dram_tensor("sdat", [128, TOTC * CHUNK], FP8,
                            kind="ExternalInput")
    id_d = nc.dram_tensor("ident", [128, 128], F16, kind="ExternalInput")
    b1_d = (nc.dram_tensor("b1bc", [128, HID], F32, kind="ExternalInput")
            if P["has_b1"] else None)
    b2_d = (nc.dram_tensor("b2bc", [128, NCLS], F32, kind="ExternalInput")
            if P["has_b2"] else None)
    y_d = nc.dram_tensor("y", [NLOC, NCLS], F32, kind="ExternalOutput")

    rg = [list(range(N_CORES))]

    with tile.TileContext(nc) as tc:
        # ---- persistent tiles ----
        perm = tc.alloc_tile_pool(name="perm", bufs=1)
        dramp = tc.alloc_tile_pool(name="dramp", bufs=1, space="DRAM")
        w1_sb = perm.tile([128, KIN * HID], F16, name="w1sb")
        w2_sb = perm.tile([128, NMAT * NCLS], F16, name="w2sb")
        dinv_sb = perm.tile([128, NW], F32, name="dinvsb")
        dinv2_sb = perm.tile([128, NW], F32, name="dinv2sb")
        idx_sb = perm.tile([128, TOTSLOTS // 16], I16, name="idxsb")
        gcnt_sb = perm.tile([1, NW * 2], mybir.dt.int32, name="gcntsb")
        id_sb = perm.tile([128, 128], F16, name="idsb")
        # on-chip g tables (dinv*h) per producer phase: [0]=lin1, [1]=hop0
        gtall = [perm.tile([128, NW * HID], F16, name=f"gtall{i}")
                 for i in range(2)]
        apart = perm.tile([128, NW * HID], F16, name="apart")
        mats0 = perm.tile([128, NW * HID], F16, name="mats0")
        logits = perm.tile([128, NW * NCLS], F32, name="logits")
        epack = perm.tile([128, NW * NCLS], F32, name="epack")
        ssum = perm.tile([128, NW], F32, name="ssum")
        lsum = perm.tile([128, NW], F32, name="lsum")
        b1_sb = perm.tile([128, HID], F32, name="b1sb") if b1_d else None
        b2_sb = perm.tile([128, NCLS], F32, name="b2sb") if b2_d else None

        # halo-exchange DRAM: per hop, A piece (local rows 0..NLH) and B
        gin = [[dramp.tile([NLH, HID], F16, name=f"gin{h}{ab}")
                for ab in "AB"] for h in range(2)]
        gout = [[dramp.tile([NN // 2, HID], F16, addr_space="Shared",
                            name=f"gout{h}{ab}")
                 for ab in "AB"] for h in range(2)]

        nc.sync.dma_start(out=w1_sb[:], in_=w1_d[:])
        nc.sync.dma_start(out=w2_sb[:], in_=w2_d[:])
        nc.sync.dma_start(out=dinv_sb[:], in_=dinv_d[:])
        nc.sync.dma_start(out=dinv2_sb[:], in_=dinv2_d[:])
        nc.sync.dma_start(out=idx_sb[:], in_=idx_d[:])
        nc.sync.dma_start(out=gcnt_sb[:], in_=gcnt_d[:])
        nc.sync.dma_start(out=id_sb[:], in_=id_d[:])
        if b1_d is not None:
            nc.sync.dma_start(out=b1_sb[:], in_=b1_d[:])
        if b2_d is not None:
            nc.sync.dma_start(out=b2_sb[:], in_=b2_d[:])

        with (
            tc.tile_pool(name="xp", bufs=3) as xp,
            tc.tile_pool(name="gp", bufs=4) as gp,
            tc.tile_pool(name="sp", bufs=4) as sp,
            tc.tile_pool(name="dp", bufs=10) as dp,
            tc.tile_pool(name="pp", bufs=1, space="PSUM") as pp,
        ):
            ACT = mybir.ActivationFunctionType

            def ggroup_write(hop, g):
                """DMA windows [4g..4g+4) of gtall[hop+1] to the gin piece
                (scalar ring: queues right behind the producing drains)."""
                gi = gin[hop + 1][0 if g < NW // 8 else 1]
                r0 = (g % (NW // 8)) * 4 * 128
                nc.scalar.dma_start(
                    out=gi[r0:r0 + 4 * 128, :]
                        .rearrange("(w p) c -> p w c", p=128),
                    in_=gtall[hop + 1][:, g * 4 * HID:(g + 1) * 4 * HID]
                        .rearrange("p (w c) -> p w c", c=HID))

            def drain_window(acc, w, hop):
                """acc: PSUM [128, HID] f32 for window w; hop 0/1/-1 (lin1).

                lin1: h = acc.  hops: h = dinv * acc (the segment sum still
                needs the dst-side dinv).  g-table for next hop = dinv * h.
                mat = relu(h); lin2 contribution streams into logits."""
                hscale = dinv_sb[:, w:w + 1] if hop >= 0 else 1.0
                gscale = (dinv2_sb[:, w:w + 1] if hop >= 0
                          else dinv_sb[:, w:w + 1])
                if hop < 1:  # produce g for the next hop's AllGather
                    nc.vector.tensor_scalar_mul(
                        gtall[hop + 1][:, w * HID:(w + 1) * HID], acc[:],
                        gscale)
                if hop == -1:
                    # minimal lin1 drain: stash relu(h); its lin2 stream
                    # runs after lin1, inside the AllGather wait window
                    nc.scalar.activation(mats0[:, w * HID:(w + 1) * HID],
                                         acc[:], ACT.Relu)
                    return
                m = dp.tile([128, HID], F16, tag="m")
                nc.scalar.activation(m[:], acc[:], ACT.Relu, scale=hscale)
                tp = pp.tile([128, 128], F16, tag="tp", bufs=2)
                nc.tensor.transpose(tp[:], m[:], id_sb[:])
                mT = dp.tile([128, 128], F16, tag="mT")
                nc.vector.tensor_copy(mT[:], tp[:])
                mi = hop + 1
                lg = pp.tile([128, NCLS], F32, tag="lg", bufs=2)
                nc.tensor.matmul(lg[:], mT[:],
                                 w2_sb[:, mi * NCLS:(mi + 1) * NCLS],
                                 start=True, stop=True)
                dst = logits[:, w * NCLS:(w + 1) * NCLS]
                if hop == -1:
                    nc.scalar.activation(dst, lg[:], ACT.Copy)
                else:
                    nc.vector.tensor_tensor(dst, dst, lg[:],
                                            op=mybir.AluOpType.add)
                if hop == 1 and b2_sb is not None:
                    nc.vector.tensor_tensor(dst, dst, b2_sb[:],
                                            op=mybir.AluOpType.add)

            def allgather(hop, ab):
                nc.gpsimd.collective_compute(
                    "AllGather", mybir.AluOpType.bypass, replica_groups=rg,
                    ins=[gin[hop][ab][:]], outs=[gout[hop][ab][:]])

            gcnt_reg = nc.gpsimd.alloc_register("gcnt_reg")

            # skipped (padded) gather slots leave stale SBUF data; zero the
            # G buffers once so the first rounds multiply 0, not garbage
            for _ in range(4):
                Gz = gp.tile([128, MAXGH * 128], F16, tag="G")
                nc.vector.memset(Gz[:], 0.0)

            # ---- lin1 ----
            for t0 in range(0, NW, XB):
                xtile = xp.tile([128, XB * KIN * 128], F16, tag="xt")
                nc.sync.dma_start(out=xtile[:], in_=xt_d[t0 // XB])
                for b in range(XB):
                    t = t0 + b
                    acc = pp.tile([128, HID], F32, tag="acc", bufs=4)
                    for k in range(KIN):
                        nc.tensor.matmul(
                            acc[:],
                            xtile[:, (b * KIN + k) * 128:(b * KIN + k + 1) * 128],
                            w1_sb[:, k * HID:(k + 1) * HID],
                            start=(k == 0), stop=(k == KIN - 1))
                    if b1_sb is not None:
                        hb = dp.tile([128, HID], F32, tag="hb")
                        nc.vector.tensor_tensor(hb[:], acc[:], b1_sb[:],
                                                op=mybir.AluOpType.add)
                        drain_window(hb, t, -1)
                    else:
                        drain_window(acc, t, -1)
                    if t % 4 == 3:
                        ggroup_write(-1, t // 4)
                        if t == NW // 2 - 1:
                            allgather(0, 0)
            allgather(0, 1)

            # deferred lin1 lin2: pure PE/DVE/scalar work that fills the
            # dead window while AllGather piece A is in flight
            for w in range(NW):
                tp = pp.tile([128, 128], F16, tag="tp", bufs=2)
                nc.tensor.transpose(tp[:], mats0[:, w * HID:(w + 1) * HID],
                                    id_sb[:])
                mT = dp.tile([128, 128], F16, tag="mT")
                nc.vector.tensor_copy(mT[:], tp[:])
                lg = pp.tile([128, NCLS], F32, tag="lg", bufs=2)
                nc.tensor.matmul(lg[:], mT[:], w2_sb[:, 0:NCLS],
                                 start=True, stop=True)
                nc.scalar.activation(logits[:, w * NCLS:(w + 1) * NCLS],
                                     lg[:], ACT.Copy)

            # ---- hops (two phases: A-side partials, then B-side finish) ----
            no_gather = os.environ.get("MIXHOP_NO_GATHER", "0") == "1"
            qc = [0]

            def gather_cols(g, ph, G3, ncols):
                """One dma_gather per (window, half), queues rotated; each
                core's tail padding is trailing -1 idxs the DGE skips."""
                colX = colA if ph == 0 else colB
                CX = CA if ph == 0 else CB
                gb0 = int(colX[g * WG])
                pb = TOTA if ph == 1 else 0
                tab = gout_cur[ph]
                for w in range(g * WG, (g + 1) * WG):
                    cw = int(CX[w])
                    if cw == 0:
                        continue
                    c0 = int(colX[w]) - gb0
                    s0 = (pb + int(colX[w])) * CHUNK
                    nc.gpsimd.reg_load(
                        gcnt_reg, gcnt_sb[0:1, w * 2 + ph:w * 2 + ph + 1])
                    nc.gpsimd.dma_gather(
                        G3[:, c0:c0 + cw, :], tab[:],
                        idx_sb[:, s0 // 16:(s0 + cw * CHUNK) // 16],
                        cw * CHUNK, gcnt_reg, HID,
                        single_packet=False,
                        queue_num=qc[0] % 4)
                    qc[0] += 1

            for hop in range(2):
                gout_cur = gout[hop]
                # ---- phase A: partial sums from A-side tables ----
                for g in range(NG):
                    na = int(NAg[g])
                    G = gp.tile([128, MAXGH * 128], F16, tag="G")
                    G3 = G[:].rearrange("p (c e) -> p c e", e=128)
                    if no_gather:
                        nc.vector.memset(G[:], 0.0)
                    else:
                        gather_cols(g, 0, G3, na)
                    sb0 = int(colA[g * WG])
                    S = sp.tile([128, MAXGH * 128], FP8, tag="S")
                    nc.sync.dma_start(
                        out=S[:, :na * 128],
                        in_=sdat_d[:, sb0 * 128:(sb0 + na) * 128])
                    for w in range(g * WG, (g + 1) * WG):
                        acc = pp.tile([128, HID], F32, tag="acc", bufs=4)
                        # self-loop: dinv[d]^2*h[d] enters as I @ g_prev
                        nc.tensor.matmul(
                            acc[:], id_sb[:],
                            gtall[hop][:, w * HID:(w + 1) * HID],
                            start=True, stop=False)
                        w0 = int(colA[w]) - sb0
                        ca = int(CA[w])
                        for c in range(ca):
                            nc.tensor.matmul(
                                acc[:], S[:, (w0 + c) * 128:(w0 + c + 1) * 128],
                                G3[:, w0 + c, :],
                                start=False, stop=(c == ca - 1))
                        nc.vector.tensor_copy(
                            apart[:, w * HID:(w + 1) * HID], acc[:])
                # ---- phase B: finish with B-side tables and drain ----
                for g in range(NG):
                    nb = int(NBg[g])
                    G = gp.tile([128, MAXGH * 128], F16, tag="G")
                    G3 = G[:].rearrange("p (c e) -> p c e", e=128)
                    if no_gather:
                        nc.vector.memset(G[:], 0.0)
                    else:
                        gather_cols(g, 1, G3, nb)
                    sb0 = int(colB[g * WG])
                    S = sp.tile([128, MAXGH * 128], FP8, tag="S")
                    nc.sync.dma_start(
                        out=S[:, :nb * 128],
                        in_=sdat_d[:, (TOTA + sb0) * 128:(TOTA + sb0 + nb) * 128])
                    for w in range(g * WG, (g + 1) * WG):
                        acc = pp.tile([128, HID], F32, tag="acc", bufs=4)
                        nc.tensor.matmul(
                            acc[:], id_sb[:],
                            apart[:, w * HID:(w + 1) * HID],
                            start=True, stop=False)
                        w0 = int(colB[w]) - sb0
                        cb = int(CB[w])
                        for c in range(cb):
                            nc.tensor.matmul(
                                acc[:], S[:, (w0 + c) * 128:(w0 + c + 1) * 128],
                                G3[:, w0 + c, :],
                                start=False, stop=(c == cb - 1))
                        drain_window(acc, w, hop)
                        if hop == 0 and w % 4 == 3:
                            ggroup_write(0, w // 4)
                    if hop == 1:
                        # fused log_softmax, batched per group; y streams out
                        w0, w1 = g * WG, (g + 1) * WG
                        lsl = logits[:, w0 * NCLS:w1 * NCLS]
                        epk = dp.tile([128, WG * NCLS], F32, tag="epk")
                        nc.scalar.activation(epk[:], lsl, ACT.Exp)
                        nc.vector.reduce_sum(
                            ssum[:, w0:w1],
                            epk[:].rearrange("p (t c) -> p t c", c=NCLS),
                            axis=mybir.AxisListType.X)
                        nc.scalar.activation(lsum[:, w0:w1], ssum[:, w0:w1],
                                             ACT.Ln)
                        for w in range(w0, w1):
                            nc.vector.tensor_scalar_sub(
                                epack[:, w * NCLS:(w + 1) * NCLS],
                                logits[:, w * NCLS:(w + 1) * NCLS],
                                lsum[:, w:w + 1])
                        nc.scalar.dma_start(
                            out=y_d[w0 * 128:w1 * 128, :]
                                .rearrange("(w p) c -> p w c", p=128),
                            in_=epack[:, w0 * NCLS:w1 * NCLS]
                                .rearrange("p (w c) -> p w c", c=NCLS))
                    # next hop's A-piece collective: launch once its drains
                    # are (about to be) done, two groups of slack for the
                    # data dep so the gpsimd queue never stalls on it
                    if hop == 0 and g == NGH + 2:
                        allgather(1, 0)
                if hop == 0:
                    allgather(1, 1)

        perm.release()
        dramp.release()
    nc.compile()
    return nc


def _ensure_ntff_hook():
    """The agent image's antenv lacks axon_hooks; synthesize it so
    run_bass_kernel_spmd(trace=True) can NTFF-profile via the axon .so."""
    import types

    if "antenv.axon_hooks" in sys.modules:
        return
    try:
        from trn_agent_boot.trn_boot import _ntff_profile_via_ctypes
        hook = _ntff_profile_via_ctypes("/opt/axon/libaxon_pjrt.so")
    except Exception:
        hook = None
    mod = types.ModuleType("antenv.axon_hooks")
    mod.get_axon_ntff_profile_hook = lambda: hook
    mod.set_axon_ntff_profile_hook = lambda h: None
    sys.modules["antenv.axon_hooks"] = mod


def kernel(x, edge_index, W1, b1, W2, b2):
    global LAST_EXEC_NS, LAST_RESULTS
    plan, in_maps = _preprocess(x, edge_index, W1, b1, W2, b2)
    nc = _build(plan)
    trace = os.environ.get("MIXHOP_TRACE", "0") == "1"
    if trace:
        _ensure_ntff_hook()
    res = run_bass_kernel_spmd(nc, in_maps, list(range(N_CORES)), trace=trace)
    LAST_EXEC_NS = res.exec_time_ns
    LAST_RESULTS = res
    out = np.concatenate([res.results[p]["y"] for p in range(N_CORES)], axis=0)
    return out.astype(np.float32)


# revision 32
# speedup vs baseline: 1.0445x; 1.0445x over previous
"""Trainium2 Bass kernel for 2-hop MixHop GCN (nn_Mixhop).

Strategy (8 NeuronCores, node sharding):
  h = x @ W1 (+b1);  GCN norm folded into row scales:
      g = dinv * h;  y[d] = dinv[d] * sum_{e: src->d} g[src]
  Per hop: the fp16 g-table is AllGathered in TWO pieces (A = local rows
  0..4095 of every core, B = rows 4096..8191) and the hop runs in two
  matching phases: phase A gathers+reduces all A-side edges into an SBUF
  partial (Apart), phase B re-injects the partial via an identity matmul
  and finishes with the B-side edges, so each ~90us collective piece
  hides behind the other phase's gather traffic.  Source rows are
  fetched per edge with gpsimd dma_gather (int16 indices address the
  32K-row A/B tables): one call per (window, half) rotated across the 4
  SWDGE queues, with each core's runtime edge count in a register so the
  trailing -1 padding idxs (chunk-ceil + max-over-cores slack, ~11% of
  descriptors) are skipped by the DGE.  Segment sums are PE matmuls
  against host-built one-hot fp8 "S" matrices (PSUM accumulation per
  128-dst window).  Self-loop edges never hit the gather path: their
  contribution dinv[d]^2*h[d] = dinv[d]*g[d] is injected by the same
  identity-matmul trick from the on-chip g tiles.  lin2 (@W2) streams
  per window at drain time into an SBUF logits accumulator; log_softmax
  at the end.

  DMA-ring discipline: the sync HWDGE ring carries only input loads
  (xt, S, weights - never blocked on compute), the scalar ring carries
  the g-table writes (queued right behind the scalar drains that
  produce them), and gathers live on the 4 SWDGE queues.
"""

import os
import sys

sys.path.insert(0, "/opt/trn_rl_repo")

import numpy as np

import concourse.bacc as bacc
import concourse.bass as bass
import concourse.mybir as mybir
import concourse.tile as tile
from concourse.bass_utils import run_bass_kernel_spmd

F32 = mybir.dt.float32
F16 = mybir.dt.float16
FP8 = mybir.dt.float8e4
I16 = mybir.dt.int16
NP_FP8 = mybir.dt.np(FP8)
NP_F16 = np.float16

N_CORES = 8
WIN = 128          # dst nodes per PSUM window
CHUNK = 128        # edges per matmul chunk
WG = 4             # windows per gather group
XB = 4             # windows per xt load

LAST_EXEC_NS = None
LAST_RESULTS = None


def _preprocess(x, edge_index, W1, b1, W2, b2):
    """Build the chunk plan (program-level constants, max over cores) and
    per-core input arrays."""
    n_nodes, d_in = x.shape
    hid = W1.shape[1]
    ncls = W2.shape[1]
    nmat = W2.shape[0] // hid
    assert n_nodes % (N_CORES * WIN) == 0
    NLOC = n_nodes // N_CORES
    NLH = NLOC // 2            # rows per A/B table piece per core
    NW = NLOC // WIN
    assert NW % WG == 0
    NG = NW // WG
    KIN = d_in // 128
    assert d_in % 128 == 0 and hid == 128

    src = np.asarray(edge_index[0], dtype=np.int64)
    dst = np.asarray(edge_index[1], dtype=np.int64)

    # degrees include the self loops even though loops bypass the gather
    deg = (np.bincount(dst, minlength=n_nodes) + 1).astype(np.float32)
    dinv = (1.0 / np.sqrt(deg)).astype(np.float32)

    core = dst // NLOC
    w_of = (dst % NLOC) // WIN
    s_r = src % NLOC
    half_of = (s_r >= NLH).astype(np.int64)   # 0 = table A, 1 = table B
    dloc = (dst % WIN).astype(np.int64)
    # int16 index into table A/B: core-major, NLH rows per core
    tab_idx = (src // NLOC) * NLH + (s_r - half_of * NLH)

    # counts per (core, window, half) -> program chunk counts = max over cores
    key = (core * NW + w_of) * 2 + half_of
    cnt = np.bincount(key, minlength=N_CORES * NW * 2).reshape(N_CORES, NW, 2)
    chunks_pc = -(-cnt // CHUNK)  # ceil-div per core
    C = chunks_pc.max(axis=0)     # [NW, 2] max over cores
    CA, CB = C[:, 0].copy(), C[:, 1].copy()
    TOTA, TOTB = int(CA.sum()), int(CB.sum())

    # per-phase group layouts: phase P columns are window-major
    NAg = np.array([CA[g * WG:(g + 1) * WG].sum() for g in range(NG)])
    NBg = np.array([CB[g * WG:(g + 1) * WG].sum() for g in range(NG)])
    gbaseA = np.concatenate([[0], np.cumsum(NAg)[:-1]])
    gbaseB = np.concatenate([[0], np.cumsum(NBg)[:-1]])
    MAXGH = int(max(NAg.max(), NBg.max()))

    colA = np.concatenate([[0], np.cumsum(CA)[:-1]])  # flat col base (A space)
    colB = np.concatenate([[0], np.cumsum(CB)[:-1]])  # flat col base (B space)

    # flat gather-slot base for (w, half): phase A slots then phase B slots
    slotbase = np.zeros((NW, 2), np.int64)
    for w in range(NW):
        slotbase[w, 0] = colA[w] * CHUNK
        slotbase[w, 1] = (TOTA + colB[w]) * CHUNK
    TOTC = TOTA + TOTB
    TOTSLOTS = TOTC * CHUNK

    one_fp8 = np.float32(1.0).astype(NP_FP8).view(np.uint8)

    plan = dict(
        n_nodes=n_nodes, NLOC=NLOC, NLH=NLH, NW=NW, NG=NG, KIN=KIN,
        hid=hid, ncls=ncls, nmat=nmat,
        CA=CA, CB=CB, NAg=NAg, NBg=NBg, gbaseA=gbaseA, gbaseB=gbaseB,
        colA=colA, colB=colB, TOTA=TOTA, TOTB=TOTB,
        TOTC=TOTC, TOTSLOTS=TOTSLOTS, MAXGH=MAXGH,
        has_b1=bool(np.any(b1 != 0)), has_b2=bool(np.any(b2 != 0)),
    )

    in_maps = []
    for p in range(N_CORES):
        sel = core == p
        t_p, w_p, h_p, dl_p = tab_idx[sel], w_of[sel], half_of[sel], dloc[sel]
        k = w_p * 2 + h_p
        order = np.argsort(k, kind="stable")
        ks = k[order]
        gcnt = np.bincount(ks, minlength=NW * 2)
        run_start = np.cumsum(gcnt) - gcnt
        run_pos = np.arange(len(ks)) - np.repeat(run_start, gcnt)
        slots = slotbase.reshape(-1)[ks] + run_pos

        idx_flat = np.full(TOTSLOTS, -1, np.int16)
        idx_flat[slots] = t_p[order].astype(np.int16)
        gcnt32 = gcnt.astype(np.int32).reshape(1, NW * 2)
        idx16 = idx_flat.reshape(TOTSLOTS // 16, 16).T  # [16, S/16]
        idx_arr = np.tile(idx16, (8, 1)).copy()         # [128, S/16]

        # S one-hot: row = pos-in-chunk, col = slot-chunk (same order as
        # the gather slots, so S col base == slot base / CHUNK * CHUNK)
        su8 = np.zeros((CHUNK, TOTC * CHUNK), np.uint8)
        pos = run_pos % CHUNK
        scol = slots - pos + dl_p[order]
        su8[pos, scol] = one_fp8
        s_arr = su8.view(NP_FP8)

        x_p = np.asarray(x[p * NLOC:(p + 1) * NLOC], dtype=np.float32)
        xt = np.ascontiguousarray(
            x_p.reshape(NW, 128, KIN, 128).transpose(0, 3, 2, 1)
            .reshape(NW // XB, XB, 128, KIN * 128).transpose(0, 2, 1, 3)
            .reshape(NW // XB, 128, XB * KIN * 128))
        dinv_p = np.ascontiguousarray(
            dinv[p * NLOC:(p + 1) * NLOC].reshape(NW, 128).T)

        m = {
            "xt": xt.astype(NP_F16),
            "w1": np.ascontiguousarray(
                np.asarray(W1, np.float32).reshape(KIN, 128, hid)
                .transpose(1, 0, 2).reshape(128, KIN * hid)).astype(NP_F16),
            "w2": np.ascontiguousarray(
                np.asarray(W2, np.float32).reshape(nmat, hid, ncls)
                .astype(NP_F16).transpose(1, 0, 2).reshape(hid, nmat * ncls)),
            "dinv": dinv_p,
            "dinv2": (dinv_p * dinv_p),
            "idx": idx_arr,
            "gcnt": gcnt32,
            "sdat": s_arr,
            "ident": np.eye(128, dtype=NP_F16),
        }
        if plan["has_b1"]:
            m["b1bc"] = np.tile(np.asarray(b1, np.float32)[None, :], (128, 1))
        if plan["has_b2"]:
            m["b2bc"] = np.tile(np.asarray(b2, np.float32)[None, :], (128, 1))
        in_maps.append(m)
    return plan, in_maps


def _build(plan):
    P = plan
    NLOC, NLH, NW, NG, KIN = P["NLOC"], P["NLH"], P["NW"], P["NG"], P["KIN"]
    HID, NCLS, NMAT = P["hid"], P["ncls"], P["nmat"]
    NN = P["n_nodes"]
    CA, CB = P["CA"], P["CB"]
    NAg, NBg = P["NAg"], P["NBg"]
    gbaseA, gbaseB = P["gbaseA"], P["gbaseB"]
    colA, colB, TOTA = P["colA"], P["colB"], P["TOTA"]
    MAXGH, TOTC, TOTSLOTS = P["MAXGH"], P["TOTC"], P["TOTSLOTS"]
    NGH = NG // 2   # groups per A/B table piece

    nc = bacc.Bacc("TRN2", target_bir_lowering=False, debug=False,
                   num_devices=N_CORES, num_swdge_queues=4)
    xt_d = nc.dram_tensor("xt", [NW // XB, 128, XB * KIN * 128], F16,
                          kind="ExternalInput")
    w1_d = nc.dram_tensor("w1", [128, KIN * HID], F16, kind="ExternalInput")
    w2_d = nc.dram_tensor("w2", [128, NMAT * NCLS], F16, kind="ExternalInput")
    dinv_d = nc.dram_tensor("dinv", [128, NW], F32, kind="ExternalInput")
    dinv2_d = nc.dram_tensor("dinv2", [128, NW], F32, kind="ExternalInput")
    idx_d = nc.dram_tensor("idx", [128, TOTSLOTS // 16], I16,
                           kind="ExternalInput")
    gcnt_d = nc.dram_tensor("gcnt", [1, NW * 2], mybir.dt.int32,
                            kind="ExternalInput")
    sdat_d = nc.dram_tensor("sdat", [128, TOTC * CHUNK], FP8,
                            kind="ExternalInput")
    id_d = nc.dram_tensor("ident", [128, 128], F16, kind="ExternalInput")
    b1_d = (nc.dram_tensor("b1bc", [128, HID], F32, kind="ExternalInput")
            if P["has_b1"] else None)
    b2_d = (nc.dram_tensor("b2bc", [128, NCLS], F32, kind="ExternalInput")
            if P["has_b2"] else None)
    y_d = nc.dram_tensor("y", [NLOC, NCLS], F32, kind="ExternalOutput")

    rg = [list(range(N_CORES))]

    with tile.TileContext(nc) as tc:
        # ---- persistent tiles ----
        perm = tc.alloc_tile_pool(name="perm", bufs=1)
        dramp = tc.alloc_tile_pool(name="dramp", bufs=1, space="DRAM")
        w1_sb = perm.tile([128, KIN * HID], F16, name="w1sb")
        w2_sb = perm.tile([128, NMAT * NCLS], F16, name="w2sb")
        dinv_sb = perm.tile([128, NW], F32, name="dinvsb")
        dinv2_sb = perm.tile([128, NW], F32, name="dinv2sb")
        idx_sb = perm.tile([128, TOTSLOTS // 16], I16, name="idxsb")
        gcnt_sb = perm.tile([1, NW * 2], mybir.dt.int32, name="gcntsb")
        id_sb = perm.tile([128, 128], F16, name="idsb")
        # on-chip g tables (dinv*h) per producer phase: [0]=lin1, [1]=hop0
        gtall = [perm.tile([128, NW * HID], F16, name=f"gtall{i}")
                 for i in range(2)]
        apart = perm.tile([128, NW * HID], F16, name="apart")
        logits = perm.tile([128, NW * NCLS], F32, name="logits")
        epack = perm.tile([128, NW * NCLS], F32, name="epack")
        ssum = perm.tile([128, NW], F32, name="ssum")
        lsum = perm.tile([128, NW], F32, name="lsum")
        b1_sb = perm.tile([128, HID], F32, name="b1sb") if b1_d else None
        b2_sb = perm.tile([128, NCLS], F32, name="b2sb") if b2_d else None

        # halo-exchange DRAM: per hop, A piece (local rows 0..NLH) and B
        gin = [[dramp.tile([NLH, HID], F16, name=f"gin{h}{ab}")
                for ab in "AB"] for h in range(2)]
        gout = [[dramp.tile([NN // 2, HID], F16, addr_space="Shared",
                            name=f"gout{h}{ab}")
                 for ab in "AB"] for h in range(2)]

        nc.sync.dma_start(out=w1_sb[:], in_=w1_d[:])
        nc.sync.dma_start(out=w2_sb[:], in_=w2_d[:])
        nc.sync.dma_start(out=dinv_sb[:], in_=dinv_d[:])
        nc.sync.dma_start(out=dinv2_sb[:], in_=dinv2_d[:])
        nc.sync.dma_start(out=idx_sb[:], in_=idx_d[:])
        nc.sync.dma_start(out=gcnt_sb[:], in_=gcnt_d[:])
        nc.sync.dma_start(out=id_sb[:], in_=id_d[:])
        if b1_d is not None:
            nc.sync.dma_start(out=b1_sb[:], in_=b1_d[:])
        if b2_d is not None:
            nc.sync.dma_start(out=b2_sb[:], in_=b2_d[:])

        with (
            tc.tile_pool(name="xp", bufs=3) as xp,
            tc.tile_pool(name="gp", bufs=4) as gp,
            tc.tile_pool(name="sp", bufs=4) as sp,
            tc.tile_pool(name="dp", bufs=10) as dp,
            tc.tile_pool(name="pp", bufs=1, space="PSUM") as pp,
        ):
            ACT = mybir.ActivationFunctionType

            def ggroup_write(hop, g):
                """DMA windows [4g..4g+4) of gtall[hop+1] to the gin piece
                (scalar ring: queues right behind the producing drains)."""
                gi = gin[hop + 1][0 if g < NW // 8 else 1]
                r0 = (g % (NW // 8)) * 4 * 128
                nc.scalar.dma_start(
                    out=gi[r0:r0 + 4 * 128, :]
                        .rearrange("(w p) c -> p w c", p=128),
                    in_=gtall[hop + 1][:, g * 4 * HID:(g + 1) * 4 * HID]
                        .rearrange("p (w c) -> p w c", c=HID))

            def drain_window(acc, w, hop):
                """acc: PSUM [128, HID] f32 for window w; hop 0/1/-1 (lin1).

                lin1: h = acc.  hops: h = dinv * acc (the segment sum still
                needs the dst-side dinv).  g-table for next hop = dinv * h.
                mat = relu(h); lin2 contribution streams into logits."""
                hscale = dinv_sb[:, w:w + 1] if hop >= 0 else 1.0
                gscale = (dinv2_sb[:, w:w + 1] if hop >= 0
                          else dinv_sb[:, w:w + 1])
                if hop < 1:  # produce g for the next hop's AllGather
                    nc.vector.tensor_scalar_mul(
                        gtall[hop + 1][:, w * HID:(w + 1) * HID], acc[:],
                        gscale)
                m = dp.tile([128, HID], F16, tag="m")
                nc.scalar.activation(m[:], acc[:], ACT.Relu, scale=hscale)
                tp = pp.tile([128, 128], F16, tag="tp", bufs=2)
                nc.tensor.transpose(tp[:], m[:], id_sb[:])
                mT = dp.tile([128, 128], F16, tag="mT")
                nc.vector.tensor_copy(mT[:], tp[:])
                mi = hop + 1
                lg = pp.tile([128, NCLS], F32, tag="lg", bufs=2)
                nc.tensor.matmul(lg[:], mT[:],
                                 w2_sb[:, mi * NCLS:(mi + 1) * NCLS],
                                 start=True, stop=True)
                dst = logits[:, w * NCLS:(w + 1) * NCLS]
                if hop == -1:
                    nc.scalar.activation(dst, lg[:], ACT.Copy)
                else:
                    nc.vector.tensor_tensor(dst, dst, lg[:],
                                            op=mybir.AluOpType.add)
                if hop == 1 and b2_sb is not None:
                    nc.vector.tensor_tensor(dst, dst, b2_sb[:],
                                            op=mybir.AluOpType.add)

            def allgather(hop, ab):
                nc.gpsimd.collective_compute(
                    "AllGather", mybir.AluOpType.bypass, replica_groups=rg,
                    ins=[gin[hop][ab][:]], outs=[gout[hop][ab][:]])

            gcnt_reg = nc.gpsimd.alloc_register("gcnt_reg")

            # skipped (padded) gather slots leave stale SBUF data; zero the
            # G buffers once so the first rounds multiply 0, not garbage
            for _ in range(4):
                Gz = gp.tile([128, MAXGH * 128], F16, tag="G")
                nc.vector.memset(Gz[:], 0.0)

            # ---- lin1 ----
            for t0 in range(0, NW, XB):
                xtile = xp.tile([128, XB * KIN * 128], F16, tag="xt")
                nc.sync.dma_start(out=xtile[:], in_=xt_d[t0 // XB])
                for b in range(XB):
                    t = t0 + b
                    acc = pp.tile([128, HID], F32, tag="acc", bufs=4)
                    for k in range(KIN):
                        nc.tensor.matmul(
                            acc[:],
                            xtile[:, (b * KIN + k) * 128:(b * KIN + k + 1) * 128],
                            w1_sb[:, k * HID:(k + 1) * HID],
                            start=(k == 0), stop=(k == KIN - 1))
                    if b1_sb is not None:
                        hb = dp.tile([128, HID], F32, tag="hb")
                        nc.vector.tensor_tensor(hb[:], acc[:], b1_sb[:],
                                                op=mybir.AluOpType.add)
                        drain_window(hb, t, -1)
                    else:
                        drain_window(acc, t, -1)
                    if t % 4 == 3:
                        ggroup_write(-1, t // 4)
                        if t == NW // 2 - 1:
                            allgather(0, 0)
            allgather(0, 1)

            # ---- hops (two phases: A-side partials, then B-side finish) ----
            no_gather = os.environ.get("MIXHOP_NO_GATHER", "0") == "1"
            qc = [0]

            def gather_cols(g, ph, G3, ncols):
                """One dma_gather per (window, half), queues rotated; each
                core's tail padding is trailing -1 idxs the DGE skips."""
                colX = colA if ph == 0 else colB
                CX = CA if ph == 0 else CB
                gb0 = int(colX[g * WG])
                pb = TOTA if ph == 1 else 0
                tab = gout_cur[ph]
                for w in range(g * WG, (g + 1) * WG):
                    cw = int(CX[w])
                    if cw == 0:
                        continue
                    c0 = int(colX[w]) - gb0
                    s0 = (pb + int(colX[w])) * CHUNK
                    nc.gpsimd.reg_load(
                        gcnt_reg, gcnt_sb[0:1, w * 2 + ph:w * 2 + ph + 1])
                    nc.gpsimd.dma_gather(
                        G3[:, c0:c0 + cw, :], tab[:],
                        idx_sb[:, s0 // 16:(s0 + cw * CHUNK) // 16],
                        cw * CHUNK, gcnt_reg, HID,
                        single_packet=False,
                        queue_num=qc[0] % 4)
                    qc[0] += 1

            for hop in range(2):
                gout_cur = gout[hop]
                # ---- phase A: partial sums from A-side tables ----
                for g in range(NG):
                    na = int(NAg[g])
                    G = gp.tile([128, MAXGH * 128], F16, tag="G")
                    G3 = G[:].rearrange("p (c e) -> p c e", e=128)
                    if no_gather:
                        nc.vector.memset(G[:], 0.0)
                    else:
                        gather_cols(g, 0, G3, na)
                    sb0 = int(colA[g * WG])
                    S = sp.tile([128, MAXGH * 128], FP8, tag="S")
                    nc.sync.dma_start(
                        out=S[:, :na * 128],
                        in_=sdat_d[:, sb0 * 128:(sb0 + na) * 128])
                    for w in range(g * WG, (g + 1) * WG):
                        acc = pp.tile([128, HID], F32, tag="acc", bufs=4)
                        # self-loop: dinv[d]^2*h[d] enters as I @ g_prev
                        nc.tensor.matmul(
                            acc[:], id_sb[:],
                            gtall[hop][:, w * HID:(w + 1) * HID],
                            start=True, stop=False)
                        w0 = int(colA[w]) - sb0
                        ca = int(CA[w])
                        for c in range(ca):
                            nc.tensor.matmul(
                                acc[:], S[:, (w0 + c) * 128:(w0 + c + 1) * 128],
                                G3[:, w0 + c, :],
                                start=False, stop=(c == ca - 1))
                        nc.vector.tensor_copy(
                            apart[:, w * HID:(w + 1) * HID], acc[:])
                # ---- phase B: finish with B-side tables and drain ----
                for g in range(NG):
                    nb = int(NBg[g])
                    G = gp.tile([128, MAXGH * 128], F16, tag="G")
                    G3 = G[:].rearrange("p (c e) -> p c e", e=128)
                    if no_gather:
                        nc.vector.memset(G[:], 0.0)
                    else:
                        gather_cols(g, 1, G3, nb)
                    sb0 = int(colB[g * WG])
                    S = sp.tile([128, MAXGH * 128], FP8, tag="S")
                    nc.sync.dma_start(
                        out=S[:, :nb * 128],
                        in_=sdat_d[:, (TOTA + sb0) * 128:(TOTA + sb0 + nb) * 128])
                    for w in range(g * WG, (g + 1) * WG):
                        acc = pp.tile([128, HID], F32, tag="acc", bufs=4)
                        nc.tensor.matmul(
                            acc[:], id_sb[:],
                            apart[:, w * HID:(w + 1) * HID],
                            start=True, stop=False)
                        w0 = int(colB[w]) - sb0
                        cb = int(CB[w])
                        for c in range(cb):
                            nc.tensor.matmul(
                                acc[:], S[:, (w0 + c) * 128:(w0 + c + 1) * 128],
                                G3[:, w0 + c, :],
                                start=False, stop=(c == cb - 1))
                        drain_window(acc, w, hop)
                        if hop == 0 and w % 4 == 3:
                            ggroup_write(0, w // 4)
                    if hop == 1 and g % 4 == 3:
                        # fused log_softmax every 4 groups (amortizes the
                        # scalar engine's Relu<->Exp<->Ln table reloads)
                        w0, w1 = (g - 3) * WG, (g + 1) * WG
                        lsl = logits[:, w0 * NCLS:w1 * NCLS]
                        epk = dp.tile([128, 4 * WG * NCLS], F32, tag="epk",
                                      bufs=2)
                        nc.scalar.activation(epk[:], lsl, ACT.Exp)
                        nc.vector.reduce_sum(
                            ssum[:, w0:w1],
                            epk[:].rearrange("p (t c) -> p t c", c=NCLS),
                            axis=mybir.AxisListType.X)
                        nc.scalar.activation(lsum[:, w0:w1], ssum[:, w0:w1],
                                             ACT.Ln)
                        for w in range(w0, w1):
                            nc.vector.tensor_scalar_sub(
                                epack[:, w * NCLS:(w + 1) * NCLS],
                                logits[:, w * NCLS:(w + 1) * NCLS],
                                lsum[:, w:w + 1])
                        nc.scalar.dma_start(
                            out=y_d[w0 * 128:w1 * 128, :]
                                .rearrange("(w p) c -> p w c", p=128),
                            in_=epack[:, w0 * NCLS:w1 * NCLS]
                                .rearrange("p (w c) -> p w c", c=NCLS))
                    # next hop's A-piece collective: launch once its drains
                    # are (about to be) done, two groups of slack for the
                    # data dep so the gpsimd queue never stalls on it
                    if hop == 0 and g == NGH + 2:
                        allgather(1, 0)
                if hop == 0:
                    allgather(1, 1)

        perm.release()
        dramp.release()
    nc.compile()
    return nc


def _ensure_ntff_hook():
    """The agent image's antenv lacks axon_hooks; synthesize it so
    run_bass_kernel_spmd(trace=True) can NTFF-profile via the axon .so."""
    import types

    if "antenv.axon_hooks" in sys.modules:
        return
    try:
        from trn_agent_boot.trn_boot import _ntff_profile_via_ctypes
        hook = _ntff_profile_via_ctypes("/opt/axon/libaxon_pjrt.so")
    except Exception:
        hook = None
    mod = types.ModuleType("antenv.axon_hooks")
    mod.get_axon_ntff_profile_hook = lambda: hook
    mod.set_axon_ntff_profile_hook = lambda h: None
    sys.modules["antenv.axon_hooks"] = mod


def kernel(x, edge_index, W1, b1, W2, b2):
    global LAST_EXEC_NS, LAST_RESULTS
    plan, in_maps = _preprocess(x, edge_index, W1, b1, W2, b2)
    nc = _build(plan)
    trace = os.environ.get("MIXHOP_TRACE", "0") == "1"
    if trace:
        _ensure_ntff_hook()
    res = run_bass_kernel_spmd(nc, in_maps, list(range(N_CORES)), trace=trace)
    LAST_EXEC_NS = res.exec_time_ns
    LAST_RESULTS = res
    out = np.concatenate([res.results[p]["y"] for p in range(N_CORES)], axis=0)
    return out.astype(np.float32)
